# revision 1
# baseline (speedup 1.0000x reference)
"""Bass kernel v2 for nn_Attention (channel attention / XCA block).

Per-core (one batch element, data-parallel over batch=8):
  qkv1 = w_qkv @ x; qkv = depthwise3x3(qkv1); q,k,v = split(qkv)
  q,k l2-normalized; G = q @ k^T per head; attn = softmax(G*temp)
  out = (w_proj @ blockdiag(attn)) @ v

v2 changes vs baseline:
- q,k path (1x1 conv + dw conv) entirely in fp8e4m3 on TensorE using
  DoubleRow perf mode (2x PE throughput; l2norm+softmax wash out the
  quantization noise: measured end-to-end rel err 0.0055 vs 0.0052 bf16).
  The 1x1 packs k=192 as [96,2]; the dw packs tap-pairs with even strides.
- q,k transposes for the gram run on the DMA XBAR (dma_start_transpose)
  instead of PE transpose + PSUM copy.
- gram packs two heads per matmul (96-col matmuls into a [96,2,96] psum G).
- v path stays bf16: 1x1 on PE; dw split: 128-ch chunk = 6 odd taps on PE
  diag + 3 even taps on DVE stt chained onto the PE partial; 64-ch chunk =
  9 taps on DVE stt (via a DMA-shifted slab copy for odd taps).
- PSUM evacuations distributed across Act/DVE/Pool engines.
"""

import sys

sys.path.insert(0, "/opt/trn_rl_repo")

import contextlib

import numpy as np
import ml_dtypes

import concourse.bass as bass
import concourse.tile as tile
from concourse import mybir
from concourse.ap import AP
from concourse.tile import add_dep_helper

BF16 = mybir.dt.bfloat16
F32 = mybir.dt.float32
F8 = mybir.dt.float8e4
NPF8 = ml_dtypes.float8_e4m3
NPBF16 = ml_dtypes.bfloat16

C = 192           # channels
OC = 3 * C        # qkv channels = 576
HEADS = 4
HC = C // HEADS   # 48
HW = 128          # image height/width
N = HW * HW       # 16384 spatial
H_STRIP = 16      # rows per strip
NSTRIP = HW // H_STRIP
OUTC = H_STRIP * 128           # dw output columns per strip
PAD = (H_STRIP + 2) * 128 + 4  # padded strip slab; data at [2, 2+(H+2)*128)
OBASE = 2 + 128                # slab offset of output row 0

# all 9 dw taps: (dy, dx), slab shift = dy*128 + dx
TAPS = [(dy, dx) for dy in (-1, 0, 1) for dx in (-1, 0, 1)]
# qk dw DoubleRow tap pairs (slab deltas; strides all even). None = zero pad.
QK_PAIRS = [(-129, -127), (-1, 1), (127, 129), (-128, 0), (126, 128)]
QK_PAIR_TAPS = [((-1, -1), (-1, 1)), ((0, -1), (0, 1)), ((1, -1), (1, 1)),
                ((-1, 0), (0, 0)), (None, (1, 0))]
V3_PE_TAPS = [(-1, -1), (-1, 1), (0, -1), (0, 1), (1, -1), (1, 1)]  # odd deltas
V3_DVE_TAPS = [(-1, 0), (0, 0), (1, 0)]                             # even deltas


def prep_weights(w_qkv, w_dw, temperature, w_proj):
    """Host-side weight layout prep. Returns dict of numpy arrays."""
    w_qkv = np.asarray(w_qkv, np.float32)
    w_dw = np.asarray(w_dw, np.float32).reshape(OC, 3, 3)
    w_proj = np.asarray(w_proj, np.float32)
    temperature = np.asarray(temperature, np.float32).reshape(HEADS)

    out = {}
    # qk 1x1 DoubleRow lhsT: [96, 2, 384] fp8; [p, t, m] = W1[m, t*96+p]
    w1qk = np.zeros((96, 2, 384), np.float32)
    for t in range(2):
        w1qk[:, t, :] = w_qkv[:384, t * 96:(t + 1) * 96].T
    out["w1qk8"] = np.ascontiguousarray(w1qk.reshape(96, 768)).astype(NPF8)

    # v 1x1 lhsT bf16: w1va [128, 192], w1vb [64, 192]; [k, m] = W1[384+m, k]
    w1vT = np.ascontiguousarray(w_qkv[384:].T)  # (192, 192)
    out["w1va"] = w1vT[:128].astype(NPBF16)
    out["w1vb"] = np.ascontiguousarray(w1vT[128:]).astype(NPBF16)

    # qk dw DoubleRow lhsT: per chunk [128, 5, 2, 128] fp8 diag pairs
    dwqk = np.zeros((128, 3, 5, 2, 128), np.float32)
    for i in range(3):
        for p, (tapA, tapB) in enumerate(QK_PAIR_TAPS):
            for t, tap in enumerate((tapA, tapB)):
                if tap is None:
                    continue
                dy, dx = tap
                w = w_dw[i * 128:(i + 1) * 128, 1 + dy, 1 + dx]
                np.fill_diagonal(dwqk[:, i, p, t, :], w)
    out["dwqk8"] = np.ascontiguousarray(dwqk.reshape(128, 3 * 5 * 2 * 128)).astype(NPF8)

    # qk fixup weights: negated fp8-rounded taps, f32: [128, 3 chunks * 6]
    wneg_qk = np.zeros((128, 18), np.float32)
    for i in range(3):
        for t, (dy, dx) in enumerate(V3_PE_TAPS):  # same 6 odd taps order
            w8 = w_dw[i * 128:(i + 1) * 128, 1 + dy, 1 + dx].astype(NPF8)
            wneg_qk[:, i * 6 + t] = -w8.astype(np.float32)
    out["wneg_qk"] = wneg_qk

    # v chunk3 (global 384..511): PE diag bf16 for 6 odd taps
    rd3 = np.zeros((128, 6 * 128), np.float32)
    for t, (dy, dx) in enumerate(V3_PE_TAPS):
        np.fill_diagonal(rd3[:, t * 128:(t + 1) * 128],
                         w_dw[384:512, 1 + dy, 1 + dx])
    out["rdiag3"] = rd3.astype(NPBF16)
    wneg3 = np.zeros((128, 6), np.float32)
    for t, (dy, dx) in enumerate(V3_PE_TAPS):
        wb = w_dw[384:512, 1 + dy, 1 + dx].astype(NPBF16)
        wneg3[:, t] = -wb.astype(np.float32)
    out["wneg3"] = wneg3
    # v chunk3 DVE taps (dy, 0): exact f32 weights
    out["wtaps3"] = np.ascontiguousarray(
        np.stack([w_dw[384:512, 1 + dy, 1] for dy in (-1, 0, 1)], 1), np.float32)

    # v chunk4 (global 512..575): 6 odd taps on PE diag bf16, 3 even on DVE
    rd4 = np.zeros((64, 6 * 64), np.float32)
    for t, (dy, dx) in enumerate(V3_PE_TAPS):
        np.fill_diagonal(rd4[:, t * 64:(t + 1) * 64],
                         w_dw[512:, 1 + dy, 1 + dx])
    out["rdiag4"] = rd4.astype(NPBF16)
    out["wtaps4"] = np.ascontiguousarray(
        np.stack([w_dw[512:, 1 + dy, 1] for dy in (-1, 0, 1)], 1), np.float32)
    wneg4 = np.zeros((64, 6), np.float32)
    for t, (dy, dx) in enumerate(V3_PE_TAPS):
        wb = w_dw[512:, 1 + dy, 1 + dx].astype(NPBF16)
        wneg4[:, t] = -wb.astype(np.float32)
    out["wneg4"] = wneg4

    # w_proj^T per head: (48, 4*192); [p, h*192+o] = w_proj[o, h*48+p]
    wpTh = np.zeros((HC, HEADS * C), np.float32)
    for h in range(HEADS):
        wpTh[:, h * C:(h + 1) * C] = w_proj[:, h * HC:(h + 1) * HC].T
    out["wpTh"] = wpTh.astype(NPBF16)

    out["temps"] = np.ascontiguousarray(
        np.broadcast_to(temperature[None, :], (HC, HEADS)), np.float32)
    out["ident48"] = np.eye(HC, dtype=np.float32)
    return out


def prep_x(x):
    """x: (B, 192, 128, 128) f32 -> per-core dicts."""
    B = x.shape[0]
    maps = []
    for b in range(B):
        xf = np.asarray(x[b], np.float32).reshape(C, N)
        x8 = np.ascontiguousarray(
            xf.reshape(2, 96, N).transpose(1, 0, 2).reshape(96, 2 * N)).astype(NPF8)
        x16 = xf.astype(NPBF16)
        maps.append({
            "x8": x8,
            "xa": np.ascontiguousarray(x16[:128]),
            "xb": np.ascontiguousarray(x16[128:]),
        })
    return maps


def build(nc):
    """Build the SPMD graph (same graph for every core)."""
    E = {}
    E["x8"] = nc.declare_dram_parameter("x8", [96, 2 * N], F8, isOutput=False)
    E["xa"] = nc.declare_dram_parameter("xa", [128, N], BF16, isOutput=False)
    E["xb"] = nc.declare_dram_parameter("xb", [64, N], BF16, isOutput=False)
    E["w1qk8"] = nc.declare_dram_parameter("w1qk8", [96, 768], F8, isOutput=False)
    E["w1va"] = nc.declare_dram_parameter("w1va", [128, 192], BF16, isOutput=False)
    E["w1vb"] = nc.declare_dram_parameter("w1vb", [64, 192], BF16, isOutput=False)
    E["dwqk8"] = nc.declare_dram_parameter("dwqk8", [128, 3 * 5 * 2 * 128], F8, isOutput=False)
    E["wneg_qk"] = nc.declare_dram_parameter("wneg_qk", [128, 18], F32, isOutput=False)
    E["rdiag3"] = nc.declare_dram_parameter("rdiag3", [128, 6 * 128], BF16, isOutput=False)
    E["wneg3"] = nc.declare_dram_parameter("wneg3", [128, 6], F32, isOutput=False)
    E["wtaps3"] = nc.declare_dram_parameter("wtaps3", [128, 3], F32, isOutput=False)
    E["rdiag4"] = nc.declare_dram_parameter("rdiag4", [64, 6 * 64], BF16, isOutput=False)
    E["wtaps4"] = nc.declare_dram_parameter("wtaps4", [64, 3], F32, isOutput=False)
    E["wneg4"] = nc.declare_dram_parameter("wneg4", [64, 6], F32, isOutput=False)
    E["wpTh"] = nc.declare_dram_parameter("wpTh", [HC, HEADS * C], BF16, isOutput=False)
    E["temps"] = nc.declare_dram_parameter("temps", [HC, HEADS], F32, isOutput=False)
    E["ident48"] = nc.declare_dram_parameter("ident48", [HC, HC], F32, isOutput=False)
    E["out"] = nc.declare_dram_parameter("out", [C, N], F32, isOutput=True)

    terminals = []

    with tile.TileContext(nc) as tc:
        with contextlib.ExitStack() as ctx:
            _build_body(ctx, tc, nc, E, terminals)

    _split_excess_waits(nc)
    return nc


def _inst_wait_cap(inst):
    return 1


def _split_excess_waits(nc, maxw_nop=1):
    """Walrus codegen rejects instructions with >1 sem wait. Move excess
    waits onto injected same-engine NoOps placed right before the offending
    instruction."""
    n_split = 0
    for f in nc.m.functions:
        for bb in f.blocks:
            insts = bb.instructions
            out = []
            changed = False
            for inst in insts:
                si = inst.sync_info
                waits = list(si.on_wait or []) if si else []
                maxw = _inst_wait_cap(inst)
                if len(waits) > maxw:
                    keep = waits[-maxw:]
                    excess = waits[:-maxw]
                    while excess:
                        grp, excess = excess[:maxw_nop], excess[maxw_nop:]
                        n_split += 1
                        nop = mybir.InstEventSemaphore(
                            name=f"wsplit_{n_split}_{inst.name}", ins=[], outs=[])
                        nop.engine = inst.engine
                        nop.debug = inst.debug
                        nop.sync_info = mybir.SyncInfo(on_wait=grp, on_update=[])
                        nc.register_instruction(nop, overwrite=True)
                        out.append(nop)
                    si.on_wait = keep
                    changed = True
                out.append(inst)
            if changed:
                bb.instructions = out


def _shift_ap(t, offset, pair_stride, n):
    """Overlapping [P, 2, n] view of tile AP t at free offset, pair stride."""
    base = t[:]
    return AP(base.tensor, base.offset + offset,
              [list(base.ap[0]), [pair_stride, 2], [1, n]])


def _build_body(ctx, tc, nc, E, terminals):
    AF = mybir.ActivationFunctionType
    ALU = mybir.AluOpType
    AX = mybir.AxisListType
    DR = mybir.MatmulPerfMode.DoubleRow

    singles = ctx.enter_context(tc.tile_pool(name="singles", bufs=1))
    vbar_pool = ctx.enter_context(tc.tile_pool(name="vbar", bufs=1))
    pQkv = ctx.enter_context(tc.tile_pool(name="pQkv", bufs=3, space="PSUM"))
    pDw = ctx.enter_context(tc.tile_pool(name="pDw", bufs=3, space="PSUM"))

    actx = contextlib.ExitStack()  # phase A+B pools; closed before phase C
    xpool = actx.enter_context(tc.tile_pool(name="xpool", bufs=2))
    slab_pool = actx.enter_context(tc.tile_pool(name="slabs", bufs=2))
    qk_pool = actx.enter_context(tc.tile_pool(name="qk", bufs=2))
    qkT_pool = actx.enter_context(tc.tile_pool(name="qkT", bufs=2))
    vmul_pool = actx.enter_context(tc.tile_pool(name="vmul", bufs=2))
    small = actx.enter_context(tc.tile_pool(name="small", bufs=2))
    pG = actx.enter_context(tc.tile_pool(name="pG", bufs=1, space="PSUM"))
    pT = actx.enter_context(tc.tile_pool(name="pT", bufs=1, space="PSUM"))

    # static evacuation engine split: Act takes 1x1 evacs (+squares),
    # DVE takes dw evacs (+v stt chains); phC alternates.
    def evac_act(dst, src):
        return nc.scalar.copy(out=dst, in_=src)

    def evac_dve(dst, src):
        return nc.vector.tensor_copy(out=dst, in_=src)

    # ---- constants ----
    w1qk8 = singles.tile([96, 2, 384], F8)
    nc.sync.dma_start(out=w1qk8[:], in_=E["w1qk8"][:].rearrange("p (t m) -> p t m", t=2))
    w1va = singles.tile([128, 192], BF16)
    nc.sync.dma_start(out=w1va[:], in_=E["w1va"][:])
    w1vb = singles.tile([64, 192], BF16)
    nc.sync.dma_start(out=w1vb[:], in_=E["w1vb"][:])
    dwqk8 = singles.tile([128, 3, 5, 2, 128], F8)
    nc.sync.dma_start(out=dwqk8[:], in_=E["dwqk8"][:].rearrange(
        "p (i q t m) -> p i q t m", i=3, q=5, t=2))
    wneg_qk = singles.tile([128, 18], F32)
    nc.sync.dma_start(out=wneg_qk[:], in_=E["wneg_qk"][:])
    rdiag3 = singles.tile([128, 6 * 128], BF16)
    nc.sync.dma_start(out=rdiag3[:], in_=E["rdiag3"][:])
    wneg3 = singles.tile([128, 6], F32)
    nc.sync.dma_start(out=wneg3[:], in_=E["wneg3"][:])
    wtaps3 = singles.tile([128, 3], F32)
    nc.sync.dma_start(out=wtaps3[:], in_=E["wtaps3"][:])
    rdiag4 = singles.tile([64, 6 * 64], BF16)
    nc.sync.dma_start(out=rdiag4[:], in_=E["rdiag4"][:])
    wtaps4 = singles.tile([64, 3], F32)
    nc.sync.dma_start(out=wtaps4[:], in_=E["wtaps4"][:])
    wneg4 = singles.tile([64, 6], F32)
    nc.sync.dma_start(out=wneg4[:], in_=E["wneg4"][:])
    wpTh = singles.tile([HC, HEADS * C], BF16)
    nc.sync.dma_start(out=wpTh[:], in_=E["wpTh"][:])
    temps = singles.tile([HC, HEADS], F32)
    nc.sync.dma_start(out=temps[:], in_=E["temps"][:])
    ident48 = singles.tile([HC, HC], F32)
    i_id = nc.sync.dma_start(out=ident48[:], in_=E["ident48"][:])
    terminals.append(i_id)

    vbar_a = vbar_pool.tile([128, N], BF16)
    vbar_b = vbar_pool.tile([64, N], BF16)
    nsq = [singles.tile([128, NSTRIP], F32, name=f"nsq{i}", tag=f"nsq{i}")
           for i in range(3)]

    # G accumulator: [96, 2, 96] f32, head-pairs packed, lives all of phase A
    G = pG.tile([96, 2, HEADS // 2 * HC], F32)

    last_pe = last_act = last_dve = last_pool = None

    # per-strip gram work emitted one strip late for pipelining
    pend_gram = None

    def emit_gram(qkT, s):
        nonlocal last_pe
        for r in range(H_STRIP):
            first = (s == 0 and r == 0)
            last = (s == NSTRIP - 1 and r == H_STRIP - 1)
            for hp in range(2):
                last_pe = nc.tensor.matmul(
                    G[:, hp, :], qkT[:, r, hp * 96:(hp + 1) * 96],
                    qkT[:, r, C + hp * 96:C + (hp + 1) * 96],
                    start=first, stop=last, skip_group_check=True)

    # ---------------- phase A: strips ----------------
    for s in range(NSTRIP):
        y0 = s * H_STRIP
        ytop = max(y0 - 1, 0)
        ybot = min(y0 + H_STRIP + 1, HW)  # exclusive
        # halo reuse: strips s>0 copy rows y0-1,y0 from the previous strip's
        # slab bottom; only rows y0+1..ybot-1 are computed fresh.
        if s == 0:
            cbase_x = 0                   # first computed x column
            wbase = 2 + 128               # slab write base
        else:
            cbase_x = (y0 + 1) * 128
            wbase = 2 + 2 * 128
        cols = ybot * 128 - cbase_x       # computed cols this strip

        prev_slabs = None if s == 0 else (slab8, slabv3, slabv4)
        slab8 = [slab_pool.tile([128, PAD], F8, name=f"slab8_{i}", tag=f"slab8_{i}")
                 for i in range(3)]
        slabv3 = slab_pool.tile([128, PAD], BF16, name="slabv3", tag="slabv3")
        slabv4 = slab_pool.tile([64, PAD], BF16, name="slabv4", tag="slabv4")
        if s < 2:
            for t_ in slab8 + [slabv3, slabv4]:
                nc.vector.memset(t_[:, 0:2], 0.0)
                nc.vector.memset(t_[:, PAD - 2:PAD], 0.0)
        if s == 0:
            for t_ in slab8 + [slabv3, slabv4]:
                nc.vector.memset(t_[:, 2:130], 0.0)  # top halo
        else:
            po8, pov3, pov4 = prev_slabs
            for dst, srcp in zip(slab8 + [slabv3, slabv4],
                                 po8 + [pov3, pov4]):
                last_pool = nc.gpsimd.tensor_copy(
                    out=dst[:, 2:258], in_=srcp[:, 2 + 16 * 128:2 + 18 * 128])

        x8_t = xpool.tile([96, 2, (H_STRIP + 1) * 128], F8, tag="x8")
        xa_t = xpool.tile([128, (H_STRIP + 1) * 128], BF16, tag="xa")
        xb_t = xpool.tile([64, (H_STRIP + 1) * 128], BF16, tag="xb")
        nc.sync.dma_start(out=x8_t[:, :, :cols], in_=E["x8"][:].rearrange(
            "p (t n) -> p t n", t=2)[:, :, cbase_x:ybot * 128])
        nc.sync.dma_start(out=xa_t[:, :cols], in_=E["xa"][:, cbase_x:ybot * 128])
        nc.sync.dma_start(out=xb_t[:, :cols], in_=E["xb"][:, cbase_x:ybot * 128])

        if s == NSTRIP - 1:
            for t_ in slab8 + [slabv3, slabv4]:
                nc.vector.memset(t_[:, 2 + (H_STRIP + 1) * 128:PAD - 2], 0.0)

        qk_sb = [qk_pool.tile([128, OUTC], BF16, name=f"qk{i}", tag=f"qk{i}")
                 for i in range(3)]

        # ---- tile emitters: A-tiles drain on Act, B-tiles on DVE ----
        def a_tile(idx):
            nonlocal last_act
            ci, base = divmod(idx * 512, ((cols + 511) // 512) * 512)
            # map flat index -> (chunk, col-base); chunks ordered qk0..2, v3, v4
            pass

        ntiles = (cols + 511) // 512

        def emit_a(ci, t):
            nonlocal last_act
            base = t * 512
            w = min(512, cols - base)
            if w <= 0:
                return
            if ci < 3:
                ps = pQkv.tile([128, 512], F32, tag="pqkv", name="psa")
                nc.tensor.matmul(
                    ps[:, :w], w1qk8[:, :, ci * 128:(ci + 1) * 128],
                    x8_t[:, :, base:base + w],
                    start=True, stop=True, perf_mode=DR)
                last_act = evac_act(
                    slab8[ci][:, wbase + base: wbase + base + w], ps[:, :w])
            else:
                mb_, msz, slab = [(0, 128, slabv3), (128, 64, slabv4)][ci - 3]
                ps = pQkv.tile([msz, 512], F32, tag="pqkv", name="psv")
                nc.tensor.matmul(ps[:, :w], w1va[:, mb_:mb_ + msz],
                                 xa_t[:, base:base + w],
                                 start=True, stop=False)
                nc.tensor.matmul(ps[:, :w], w1vb[:, mb_:mb_ + msz],
                                 xb_t[:, base:base + w],
                                 start=False, stop=True)
                last_act = evac_act(slab[:, wbase + base: wbase + base + w],
                                    ps[:, :w])

        def emit_b(bi, nt):
            nonlocal last_act, last_dve, last_pe
            if bi < 3:  # qk dw chunk bi
                i = bi
                ps = pDw.tile([128, 512], F32, tag="pdw", name="psd")
                for p, (dA, dB) in enumerate(QK_PAIRS):
                    rhs = _shift_ap(slab8[i], OBASE + nt * 512 + dA, dB - dA, 512)
                    nc.tensor.matmul(ps[:], dwqk8[:, i, p, :, :], rhs,
                                     start=(p == 0), stop=(p == 4),
                                     perf_mode=DR)
                dst = qk_sb[i][:, nt * 512:(nt + 1) * 512]
                if nt % 4 == 3:
                    last_act = evac_act(dst, ps[:])
                else:
                    last_dve = evac_dve(dst, ps[:])
            else:  # v dw odd taps: bi==3 -> chunk3, bi==4 -> chunk4
                vb = vbar_a if bi == 3 else vbar_b
                rd = rdiag3 if bi == 3 else rdiag4
                csz = 128 if bi == 3 else 64
                slab = slabv3 if bi == 3 else slabv4
                ps = pDw.tile([csz, 512], F32, tag="pdw", name="psd")
                for t, (dy, dx) in enumerate(V3_PE_TAPS):
                    d = dy * 128 + dx
                    o = OBASE + nt * 512 + d
                    last_pe = nc.tensor.matmul(
                        ps[:], rd[:, t * csz:(t + 1) * csz],
                        slab[:, o:o + 512],
                        start=(t == 0), stop=(t == 5))
                dst = vb[:, y0 * 128 + nt * 512: y0 * 128 + (nt + 1) * 512]
                if nt % 4 == 3:
                    last_act = evac_act(dst, ps[:])
                else:
                    last_dve = evac_dve(dst, ps[:])

        # ---- zipper: interleave A (1x1) and B (dw) tile emission ----
        A = [(ci, t) for ci in range(5) for t in range(ntiles)]
        B = [(bi, nt) for bi in range(5) for nt in range(OUTC // 512)]
        DELAY = 6
        ai = bi_ = 0
        emitted_gram = False
        while ai < len(A) or bi_ < len(B):
            if ai < len(A):
                emit_a(*A[ai]); ai += 1
            if ai >= len(A) and not emitted_gram and pend_gram is not None:
                emit_gram(*pend_gram)
                pend_gram = None
                emitted_gram = True
            if ai >= DELAY or ai >= len(A):
                if bi_ < len(B) and (bi_ < (ai - DELAY) + 1 or ai >= len(A)):
                    emit_b(*B[bi_]); bi_ += 1
        if pend_gram is not None:
            emit_gram(*pend_gram)
            pend_gram = None

        # --- qk x-edge fixups (6 odd taps wrap at x=0/127) on DVE ---
        for i in range(3):
            d3 = qk_sb[i].rearrange("p (r x) -> p r x", x=128)
            dst_c0 = d3[:, :, 0:1]
            dst_c127 = d3[:, :, 127:128]
            for dy in (-1, 0, 1):
                t_m1 = V3_PE_TAPS.index((dy, -1))
                t_p1 = V3_PE_TAPS.index((dy, 1))
                a0 = (1 + dy) * 128 + 1
                src0 = slab8[i][:, a0:a0 + OUTC].rearrange(
                    "p (r x) -> p r x", x=128)[:, :, 0:1]
                last_dve = nc.vector.scalar_tensor_tensor(
                    out=dst_c0, in0=src0,
                    scalar=wneg_qk[:, i * 6 + t_m1: i * 6 + t_m1 + 1],
                    in1=dst_c0, op0=ALU.mult, op1=ALU.add)
                a = (dy + 1) * 128 + 4
                src1 = slab8[i][:, a:a + OUTC].rearrange(
                    "p (r x) -> p r x", x=128)[:, :, 126:127]
                last_dve = nc.vector.scalar_tensor_tensor(
                    out=dst_c127, in0=src1,
                    scalar=wneg_qk[:, i * 6 + t_p1: i * 6 + t_p1 + 1],
                    in1=dst_c127, op0=ALU.mult, op1=ALU.add)

        # --- norms: sum of squares per channel on Act ---
        sq_scr = vmul_pool.tile([128, OUTC], BF16, tag="vp0", name="sq_scr")
        for i in range(3):
            last_act = nc.scalar.activation(
                out=sq_scr[:], in_=qk_sb[i][:], func=AF.Square,
                accum_out=nsq[i][:, s:s + 1])

        # --- qk transpose via DMA XBAR into [x, r, ch] ---
        qkT = qkT_pool.tile([128, H_STRIP, 2 * C], BF16, tag="qkT")
        for i in range(3):
            nc.sync.dma_start_transpose(
                out=qkT[:, :, i * 128:(i + 1) * 128], in_=qk_sb[i][:])
        pend_gram = (qkT, s)

        # --- v even taps: mul (4x) into scratch, add (2x) in-place to vbar ---
        va_sl = vbar_a[:, y0 * 128: y0 * 128 + OUTC]
        vb_sl = vbar_b[:, y0 * 128: y0 * 128 + OUTC]
        for ci, (slab, wt, sl, csz) in enumerate(
                [(slabv3, wtaps3, va_sl, 128), (slabv4, wtaps4, vb_sl, 64)]):
            for j, dy in enumerate((-1, 0, 1)):
                o = OBASE + dy * 128
                pr = vmul_pool.tile([csz, OUTC], BF16, tag=f"vp{j % 2}",
                                    name="pr")
                last_dve = nc.vector.tensor_scalar_mul(
                    pr[:], slab[:, o:o + OUTC], wt[:, j:j + 1])
                last_dve = nc.vector.tensor_add(sl, sl, pr[:])

        # --- v fixups (PE odd taps wrap) on DVE ---
        for slab, wneg, dst, csz in [(slabv3, wneg3, va_sl, 128),
                                     (slabv4, wneg4, vb_sl, 64)]:
            d3 = dst.rearrange("p (r x) -> p r x", x=128)
            for dy in (-1, 0, 1):
                t_m1 = V3_PE_TAPS.index((dy, -1))
                t_p1 = V3_PE_TAPS.index((dy, 1))
                a0 = (1 + dy) * 128 + 1
                src0 = slab[:, a0:a0 + OUTC].rearrange(
                    "p (r x) -> p r x", x=128)[:, :, 0:1]
                last_dve = nc.vector.scalar_tensor_tensor(
                    out=d3[:, :, 0:1], in0=src0,
                    scalar=wneg[:, t_m1:t_m1 + 1],
                    in1=d3[:, :, 0:1], op0=ALU.mult, op1=ALU.add)
                a = (dy + 1) * 128 + 4
                src1 = slab[:, a:a + OUTC].rearrange(
                    "p (r x) -> p r x", x=128)[:, :, 126:127]
                last_dve = nc.vector.scalar_tensor_tensor(
                    out=d3[:, :, 127:128], in0=src1,
                    scalar=wneg[:, t_p1:t_p1 + 1],
                    in1=d3[:, :, 127:128], op0=ALU.mult, op1=ALU.add)

    # tail gram
    if pend_gram is not None:
        emit_gram(*pend_gram)
        pend_gram = None

    # ---------------- phase B ----------------
    Gsb = small.tile([96, 2, 96], F32, tag="gsb")
    last_act = nc.scalar.copy(out=Gsb[:], in_=G[:])

    rn = []
    for i in range(3):
        tot = small.tile([128, 1], F32, tag=f"tot{i}")
        nc.vector.tensor_reduce(out=tot[:], in_=nsq[i][:], axis=AX.X, op=ALU.add)
        rt = small.tile([128, 1], F32, tag=f"rt{i}")
        nc.scalar.sqrt(out=rt[:], in_=tot[:])
        rr = small.tile([128, 1], F32, tag=f"rr{i}")
        nc.vector.reciprocal(out=rr[:], in_=rt[:])
        rn.append(rr)

    def gather_head(dst, global_base):
        done = 0
        g = global_base
        while done < HC:
            oc, off = g // 128, g % 128
            take = min(HC - done, 128 - off)
            nc.sync.dma_start(out=dst[done:done + take, :],
                              in_=rn[oc][off:off + take, :])
            done += take
            g += take

    mh_sb = []
    for h in range(HEADS):
        hp, off = h // 2, (h % 2) * HC
        rq = small.tile([HC, 1], F32, tag="rq")
        gather_head(rq, h * HC)
        rk = small.tile([HC, 1], F32, tag="rk")
        gather_head(rk, C + h * HC)
        rqt = small.tile([HC, 1], F32, tag="rqt")
        nc.vector.tensor_mul(rqt[:], rq[:], temps[:, h:h + 1])
        g_h = small.tile([HC, HC], F32, tag="gh")
        nc.sync.dma_start(out=g_h[:], in_=Gsb[off:off + HC, hp, off:off + HC])
        z1 = small.tile([HC, HC], F32, tag="z1")
        nc.vector.tensor_scalar_mul(z1[:], g_h[:], rqt[:])
        z1T_ps = pT.tile([HC, HC], F32, tag="ptz")
        nc.tensor.transpose(z1T_ps[:], z1[:], ident48[:])
        z1T = small.tile([HC, HC], F32, tag="z1T")
        nc.scalar.copy(out=z1T[:], in_=z1T_ps[:])
        z2 = small.tile([HC, HC], F32, tag="z2")
        nc.vector.tensor_scalar_mul(z2[:], z1T[:], rk[:])
        z2T_ps = pT.tile([HC, HC], F32, tag="ptz")
        nc.tensor.transpose(z2T_ps[:], z2[:], ident48[:])
        z = small.tile([HC, HC], F32, tag="z")
        nc.scalar.copy(out=z[:], in_=z2T_ps[:])
        m = small.tile([HC, 1], F32, tag="m")
        nc.vector.reduce_max(m[:], z[:], AX.X)
        nm = small.tile([HC, 1], F32, tag="nm")
        nc.vector.tensor_scalar_mul(nm[:], m[:], -1.0)
        e = small.tile([HC, HC], F32, tag="e")
        nc.scalar.activation(out=e[:], in_=z[:], func=AF.Exp, bias=nm[:], scale=1.0)
        ssum = small.tile([HC, 1], F32, tag="ssum")
        nc.vector.reduce_sum(ssum[:], e[:], AX.X)
        rs = small.tile([HC, 1], F32, tag="rs")
        nc.vector.reciprocal(rs[:], ssum[:])
        attn = small.tile([HC, HC], BF16, tag="attn")
        last_dve = nc.vector.tensor_scalar_mul(attn[:], e[:], rs[:])
        mh = pDw.tile([HC, C], F32, tag="pdw")
        nc.tensor.matmul(mh[:], attn[:], wpTh[:, h * C:(h + 1) * C],
                         start=True, stop=True)
        msb = small.tile([HC, C], BF16, tag=f"msb{h}")
        nc.scalar.copy(out=msb[:], in_=mh[:])
        mh_sb.append(msb)

    MTa = singles.tile([128, C], BF16)
    MTb = singles.tile([64, C], BF16)
    nc.sync.dma_start(out=MTa[0:48, :], in_=mh_sb[0][:])
    nc.sync.dma_start(out=MTa[48:96, :], in_=mh_sb[1][:])
    nc.sync.dma_start(out=MTa[96:128, :], in_=mh_sb[2][0:32, :])
    nc.sync.dma_start(out=MTb[0:16, :], in_=mh_sb[2][32:48, :])
    i_m = nc.sync.dma_start(out=MTb[16:64, :], in_=mh_sb[3][:])
    terminals.append(i_m)

    # ---------------- phase C: out = blockdiag-attn-proj @ vbar ----------------
    actx.close()  # free phase-A SBUF for wide output staging
    outp = ctx.enter_context(tc.tile_pool(name="outp", bufs=2))
    BLK = 2048
    for blk in range(N // BLK):
        o0 = outp.tile([128, BLK], F32, tag="o0")
        o1 = outp.tile([64, BLK], F32, tag="o1")
        for j in range(BLK // 512):
            nt = blk * (BLK // 512) + j
            sl = slice(nt * 512, (nt + 1) * 512)
            ps0 = pQkv.tile([128, 512], F32, tag="pqkv")
            nc.tensor.matmul(ps0[:], MTa[:, 0:128], vbar_a[:, sl], start=True, stop=False)
            nc.tensor.matmul(ps0[:], MTb[:, 0:128], vbar_b[:, sl], start=False, stop=True)
            ps1 = pDw.tile([64, 512], F32, tag="pdw")
            nc.tensor.matmul(ps1[:], MTa[:, 128:192], vbar_a[:, sl], start=True, stop=False)
            last_pe = nc.tensor.matmul(ps1[:], MTb[:, 128:192], vbar_b[:, sl],
                                       start=False, stop=True)
            last_act = nc.scalar.copy(out=o0[:, j * 512:(j + 1) * 512], in_=ps0[:])
            last_dve = nc.vector.tensor_copy(out=o1[:, j * 512:(j + 1) * 512], in_=ps1[:])
        i0 = nc.sync.dma_start(out=E["out"][0:128, blk * BLK:(blk + 1) * BLK], in_=o0[:])
        i1 = nc.gpsimd.dma_start(out=E["out"][128:192, blk * BLK:(blk + 1) * BLK], in_=o1[:])
        terminals.append(i0)
        terminals.append(i1)

    terminals.append(last_pe)
    terminals.append(last_act)
    terminals.append(last_dve)
    terminals.append(last_pool)


# ----------------------------------------------------------------------------
# Public entry point: full inputs -> full output, 8-way data-parallel over
# batch across NeuronCores 0-7.
# ----------------------------------------------------------------------------

def kernel(x, w_qkv, w_dw, temperature, w_proj):
    from concourse.bass_utils import run_bass_kernel_spmd

    x = np.asarray(x, np.float32)
    B = x.shape[0]
    assert x.shape == (8, C, HW, HW), x.shape

    nc = bass.Bass()
    build(nc)

    wmaps = prep_weights(w_qkv, w_dw, temperature, w_proj)
    xmaps = prep_x(x)
    in_maps = [{**wmaps, **xm} for xm in xmaps]

    res = run_bass_kernel_spmd(nc, in_maps, core_ids=list(range(8)))
    out = np.stack([np.asarray(res.results[b]["out"], np.float32)
                    .reshape(C, HW, HW) for b in range(B)])
    return out



# revision 22
# speedup vs baseline: 1.3704x; 1.3704x over previous
"""Bass kernel v2 for nn_Attention (channel attention / XCA block).

Per-core (one batch element, data-parallel over batch=8):
  qkv1 = w_qkv @ x; qkv = depthwise3x3(qkv1); q,k,v = split(qkv)
  q,k l2-normalized; G = q @ k^T per head; attn = softmax(G*temp)
  out = (w_proj @ blockdiag(attn)) @ v

v2 changes vs baseline:
- q,k path (1x1 conv + dw conv) entirely in fp8e4m3 on TensorE using
  DoubleRow perf mode (2x PE throughput; l2norm+softmax wash out the
  quantization noise: measured end-to-end rel err 0.0055 vs 0.0052 bf16).
  The 1x1 packs k=192 as [96,2]; the dw packs tap-pairs with even strides.
- q,k transposes for the gram run on the DMA XBAR (dma_start_transpose)
  instead of PE transpose + PSUM copy.
- gram packs two heads per matmul (96-col matmuls into a [96,2,96] psum G).
- v path stays bf16: 1x1 on PE; dw split: 128-ch chunk = 6 odd taps on PE
  diag + 3 even taps on DVE stt chained onto the PE partial; 64-ch chunk =
  9 taps on DVE stt (via a DMA-shifted slab copy for odd taps).
- PSUM evacuations distributed across Act/DVE/Pool engines.
"""

import sys

sys.path.insert(0, "/opt/trn_rl_repo")

import contextlib

import numpy as np
import ml_dtypes

import concourse.bass as bass
import concourse.tile as tile
from concourse import mybir
from concourse.ap import AP
from concourse.tile import add_dep_helper

BF16 = mybir.dt.bfloat16
F32 = mybir.dt.float32
F8 = mybir.dt.float8e4
NPF8 = ml_dtypes.float8_e4m3
NPBF16 = ml_dtypes.bfloat16

C = 192           # channels
OC = 3 * C        # qkv channels = 576
HEADS = 4
HC = C // HEADS   # 48
HW = 128          # image height/width
N = HW * HW       # 16384 spatial
H_STRIP = 16      # rows per strip
NSTRIP = HW // H_STRIP
OUTC = H_STRIP * 128           # dw output columns per strip
PAD = (H_STRIP + 2) * 128 + 4  # padded strip slab; data at [2, 2+(H+2)*128)
OBASE = 2 + 128                # slab offset of output row 0

# all 9 dw taps: (dy, dx), slab shift = dy*128 + dx
TAPS = [(dy, dx) for dy in (-1, 0, 1) for dx in (-1, 0, 1)]
# qk dw DoubleRow tap pairs (slab deltas; strides all even). Center (0,0)
# is applied during the psum evacuation (DVE stt with the fp8 slab).
QK_PAIRS = [(-129, -127), (-1, 1), (127, 129), (-128, 128)]
QK_PAIR_TAPS = [((-1, -1), (-1, 1)), ((0, -1), (0, 1)), ((1, -1), (1, 1)),
                ((-1, 0), (1, 0))]
V3_PE_TAPS = [(-1, -1), (-1, 1), (0, -1), (0, 1), (1, -1), (1, 1)]  # odd deltas
V3_DVE_TAPS = [(-1, 0), (0, 0), (1, 0)]                             # even deltas


def prep_weights(w_qkv, w_dw, temperature, w_proj):
    """Host-side weight layout prep. Returns dict of numpy arrays."""
    w_qkv = np.asarray(w_qkv, np.float32)
    w_dw = np.asarray(w_dw, np.float32).reshape(OC, 3, 3)
    w_proj = np.asarray(w_proj, np.float32)
    temperature = np.asarray(temperature, np.float32).reshape(HEADS)

    out = {}
    # qk 1x1 DoubleRow lhsT: [96, 2, 384] fp8; [p, t, m] = W1[m, t*96+p]
    w1qk = np.zeros((96, 2, 384), np.float32)
    for t in range(2):
        w1qk[:, t, :] = w_qkv[:384, t * 96:(t + 1) * 96].T
    out["w1qk8"] = np.ascontiguousarray(w1qk.reshape(96, 768)).astype(NPF8)

    # v 1x1 lhsT bf16: w1va [128, 192], w1vb [64, 192]; [k, m] = W1[384+m, k]
    w1vT = np.ascontiguousarray(w_qkv[384:].T)  # (192, 192)
    out["w1va"] = w1vT[:128].astype(NPBF16)
    out["w1vb"] = np.ascontiguousarray(w1vT[128:]).astype(NPBF16)

    # qk dw DoubleRow lhsT: per chunk [128, 4, 2, 128] fp8 diag pairs
    dwqk = np.zeros((128, 3, 4, 2, 128), np.float32)
    for i in range(3):
        for p, (tapA, tapB) in enumerate(QK_PAIR_TAPS):
            for t, tap in enumerate((tapA, tapB)):
                if tap is None:
                    continue
                dy, dx = tap
                w = w_dw[i * 128:(i + 1) * 128, 1 + dy, 1 + dx]
                np.fill_diagonal(dwqk[:, i, p, t, :], w)
    out["dwqk8"] = np.ascontiguousarray(dwqk.reshape(128, 3 * 4 * 2 * 128)).astype(NPF8)
    # qk center-tap weights (exact f32, applied at evac): [128, 3]
    out["wc_qk"] = np.ascontiguousarray(
        np.stack([w_dw[i * 128:(i + 1) * 128, 1, 1] for i in range(3)], 1),
        np.float32)

    # qk fixup weights: negated fp8-rounded taps, f32: [128, 3 chunks * 6]
    wneg_qk = np.zeros((128, 18), np.float32)
    for i in range(3):
        for t, (dy, dx) in enumerate(V3_PE_TAPS):  # same 6 odd taps order
            w8 = w_dw[i * 128:(i + 1) * 128, 1 + dy, 1 + dx].astype(NPF8)
            wneg_qk[:, i * 6 + t] = -w8.astype(np.float32)
    out["wneg_qk"] = wneg_qk

    # v chunk3 (global 384..511): PE diag bf16 for 6 odd taps
    rd3 = np.zeros((128, 6 * 128), np.float32)
    for t, (dy, dx) in enumerate(V3_PE_TAPS):
        np.fill_diagonal(rd3[:, t * 128:(t + 1) * 128],
                         w_dw[384:512, 1 + dy, 1 + dx])
    out["rdiag3"] = rd3.astype(NPBF16)
    wneg3 = np.zeros((128, 6), np.float32)
    for t, (dy, dx) in enumerate(V3_PE_TAPS):
        wb = w_dw[384:512, 1 + dy, 1 + dx].astype(NPBF16)
        wneg3[:, t] = -wb.astype(np.float32)
    out["wneg3"] = wneg3
    # v chunk3 DVE taps (dy, 0): exact f32 weights
    out["wtaps3"] = np.ascontiguousarray(
        np.stack([w_dw[384:512, 1 + dy, 1] for dy in (-1, 0, 1)], 1), np.float32)

    # v chunk4 (global 512..575): 6 odd taps on PE diag bf16, 3 even on DVE
    rd4 = np.zeros((64, 6 * 64), np.float32)
    for t, (dy, dx) in enumerate(V3_PE_TAPS):
        np.fill_diagonal(rd4[:, t * 64:(t + 1) * 64],
                         w_dw[512:, 1 + dy, 1 + dx])
    out["rdiag4"] = rd4.astype(NPBF16)
    out["wtaps4"] = np.ascontiguousarray(
        np.stack([w_dw[512:, 1 + dy, 1] for dy in (-1, 0, 1)], 1), np.float32)
    wneg4 = np.zeros((64, 6), np.float32)
    for t, (dy, dx) in enumerate(V3_PE_TAPS):
        wb = w_dw[512:, 1 + dy, 1 + dx].astype(NPBF16)
        wneg4[:, t] = -wb.astype(np.float32)
    out["wneg4"] = wneg4

    # w_proj^T per head: (48, 4*192); [p, h*192+o] = w_proj[o, h*48+p]
    wpTh = np.zeros((HC, HEADS * C), np.float32)
    for h in range(HEADS):
        wpTh[:, h * C:(h + 1) * C] = w_proj[:, h * HC:(h + 1) * HC].T
    out["wpTh"] = wpTh.astype(NPBF16)

    out["temps"] = np.ascontiguousarray(
        np.broadcast_to(temperature[None, :], (HC, HEADS)), np.float32)
    out["ident48"] = np.eye(HC, dtype=np.float32)
    return out


def prep_x(x):
    """x: (B, 192, 128, 128) f32 -> per-core dicts."""
    B = x.shape[0]
    maps = []
    for b in range(B):
        xf = np.asarray(x[b], np.float32).reshape(C, N)
        x8 = np.ascontiguousarray(
            xf.reshape(2, 96, N).transpose(1, 0, 2).reshape(96, 2 * N)).astype(NPF8)
        x16 = xf.astype(NPBF16)
        maps.append({
            "x8": x8,
            "xa": np.ascontiguousarray(x16[:128]),
            "xb": np.ascontiguousarray(x16[128:]),
        })
    return maps


def build(nc):
    """Build the SPMD graph (same graph for every core)."""
    E = {}
    E["x8"] = nc.declare_dram_parameter("x8", [96, 2 * N], F8, isOutput=False)
    E["xa"] = nc.declare_dram_parameter("xa", [128, N], BF16, isOutput=False)
    E["xb"] = nc.declare_dram_parameter("xb", [64, N], BF16, isOutput=False)
    E["w1qk8"] = nc.declare_dram_parameter("w1qk8", [96, 768], F8, isOutput=False)
    E["w1va"] = nc.declare_dram_parameter("w1va", [128, 192], BF16, isOutput=False)
    E["w1vb"] = nc.declare_dram_parameter("w1vb", [64, 192], BF16, isOutput=False)
    E["dwqk8"] = nc.declare_dram_parameter("dwqk8", [128, 3 * 4 * 2 * 128], F8, isOutput=False)
    E["wc_qk"] = nc.declare_dram_parameter("wc_qk", [128, 3], F32, isOutput=False)
    E["wneg_qk"] = nc.declare_dram_parameter("wneg_qk", [128, 18], F32, isOutput=False)
    E["rdiag3"] = nc.declare_dram_parameter("rdiag3", [128, 6 * 128], BF16, isOutput=False)
    E["wneg3"] = nc.declare_dram_parameter("wneg3", [128, 6], F32, isOutput=False)
    E["wtaps3"] = nc.declare_dram_parameter("wtaps3", [128, 3], F32, isOutput=False)
    E["rdiag4"] = nc.declare_dram_parameter("rdiag4", [64, 6 * 64], BF16, isOutput=False)
    E["wtaps4"] = nc.declare_dram_parameter("wtaps4", [64, 3], F32, isOutput=False)
    E["wneg4"] = nc.declare_dram_parameter("wneg4", [64, 6], F32, isOutput=False)
    E["wpTh"] = nc.declare_dram_parameter("wpTh", [HC, HEADS * C], BF16, isOutput=False)
    E["temps"] = nc.declare_dram_parameter("temps", [HC, HEADS], F32, isOutput=False)
    E["ident48"] = nc.declare_dram_parameter("ident48", [HC, HC], F32, isOutput=False)
    E["out"] = nc.declare_dram_parameter("out", [C, N], BF16, isOutput=True)

    terminals = []

    with tile.TileContext(nc) as tc:
        with contextlib.ExitStack() as ctx:
            _build_body(ctx, tc, nc, E, terminals)

    _split_excess_waits(nc)
    return nc


def _inst_wait_cap(inst):
    return 1


def _split_excess_waits(nc, maxw_nop=1):
    """Walrus codegen rejects instructions with >1 sem wait. Move excess
    waits onto injected same-engine NoOps placed right before the offending
    instruction."""
    n_split = 0
    for f in nc.m.functions:
        for bb in f.blocks:
            insts = bb.instructions
            out = []
            changed = False
            for inst in insts:
                si = inst.sync_info
                waits = list(si.on_wait or []) if si else []
                maxw = _inst_wait_cap(inst)
                if len(waits) > maxw:
                    keep = waits[-maxw:]
                    excess = waits[:-maxw]
                    while excess:
                        grp, excess = excess[:maxw_nop], excess[maxw_nop:]
                        n_split += 1
                        nop = mybir.InstEventSemaphore(
                            name=f"wsplit_{n_split}_{inst.name}", ins=[], outs=[])
                        nop.engine = inst.engine
                        nop.debug = inst.debug
                        nop.sync_info = mybir.SyncInfo(on_wait=grp, on_update=[])
                        nc.register_instruction(nop, overwrite=True)
                        out.append(nop)
                    si.on_wait = keep
                    changed = True
                out.append(inst)
            if changed:
                bb.instructions = out


def _shift_ap(t, offset, pair_stride, n):
    """Overlapping [P, 2, n] view of tile AP t at free offset, pair stride."""
    base = t[:]
    return AP(base.tensor, base.offset + offset,
              [list(base.ap[0]), [pair_stride, 2], [1, n]])


def _build_body(ctx, tc, nc, E, terminals):
    AF = mybir.ActivationFunctionType
    ALU = mybir.AluOpType
    AX = mybir.AxisListType
    DR = mybir.MatmulPerfMode.DoubleRow

    singles = ctx.enter_context(tc.tile_pool(name="singles", bufs=1))
    vbar_pool = ctx.enter_context(tc.tile_pool(name="vbar", bufs=1))

    actx = contextlib.ExitStack()  # phase A+B pools; closed before phase C
    pQkv = actx.enter_context(tc.tile_pool(name="pQkv", bufs=2, space="PSUM"))
    pDw = actx.enter_context(tc.tile_pool(name="pDw", bufs=2, space="PSUM"))
    xpool = actx.enter_context(tc.tile_pool(name="xpool", bufs=2))
    slab_pool = actx.enter_context(tc.tile_pool(name="slabs", bufs=2))
    qk_pool = actx.enter_context(tc.tile_pool(name="qk", bufs=2))
    qkT_pool = actx.enter_context(tc.tile_pool(name="qkT", bufs=2))
    vmul_pool = actx.enter_context(tc.tile_pool(name="vmul", bufs=2))
    small = actx.enter_context(tc.tile_pool(name="small", bufs=2))
    pG = actx.enter_context(tc.tile_pool(name="pG", bufs=1, space="PSUM"))
    pT = actx.enter_context(tc.tile_pool(name="pT", bufs=1, space="PSUM"))

    # static evacuation engine split: Act takes 1x1 evacs (+squares),
    # DVE takes dw evacs (+v stt chains); phC alternates.
    def evac_act(dst, src):
        return nc.scalar.copy(out=dst, in_=src)

    def evac_dve(dst, src):
        return nc.vector.tensor_copy(out=dst, in_=src)

    # ---- constants ----
    w1qk8 = singles.tile([96, 2, 384], F8)
    nc.sync.dma_start(out=w1qk8[:], in_=E["w1qk8"][:].rearrange("p (t m) -> p t m", t=2))
    w1va = singles.tile([128, 192], BF16)
    nc.sync.dma_start(out=w1va[:], in_=E["w1va"][:])
    w1vb = singles.tile([64, 192], BF16)
    nc.sync.dma_start(out=w1vb[:], in_=E["w1vb"][:])
    dwqk8 = singles.tile([128, 3, 4, 2, 128], F8)
    nc.sync.dma_start(out=dwqk8[:], in_=E["dwqk8"][:].rearrange(
        "p (i q t m) -> p i q t m", i=3, q=4, t=2))
    wc_qk = singles.tile([128, 3], F32)
    nc.sync.dma_start(out=wc_qk[:], in_=E["wc_qk"][:])
    wneg_qk = singles.tile([128, 18], F32)
    nc.sync.dma_start(out=wneg_qk[:], in_=E["wneg_qk"][:])
    rdiag3 = singles.tile([128, 6 * 128], BF16)
    nc.sync.dma_start(out=rdiag3[:], in_=E["rdiag3"][:])
    wneg3 = singles.tile([128, 6], F32)
    nc.sync.dma_start(out=wneg3[:], in_=E["wneg3"][:])
    wtaps3 = singles.tile([128, 3], F32)
    nc.sync.dma_start(out=wtaps3[:], in_=E["wtaps3"][:])
    rdiag4 = singles.tile([64, 6 * 64], BF16)
    nc.sync.dma_start(out=rdiag4[:], in_=E["rdiag4"][:])
    wtaps4 = singles.tile([64, 3], F32)
    nc.sync.dma_start(out=wtaps4[:], in_=E["wtaps4"][:])
    wneg4 = singles.tile([64, 6], F32)
    nc.sync.dma_start(out=wneg4[:], in_=E["wneg4"][:])
    wpTh = singles.tile([HC, HEADS * C], BF16)
    nc.sync.dma_start(out=wpTh[:], in_=E["wpTh"][:])
    temps = singles.tile([HC, HEADS], F32)
    nc.sync.dma_start(out=temps[:], in_=E["temps"][:])
    ident48 = singles.tile([HC, HC], F32)
    i_id = nc.sync.dma_start(out=ident48[:], in_=E["ident48"][:])
    terminals.append(i_id)

    vbar_a = vbar_pool.tile([128, N], BF16)
    vbar_b = vbar_pool.tile([64, N], BF16)
    nsq = [singles.tile([128, NSTRIP], F32, name=f"nsq{i}", tag=f"nsq{i}")
           for i in range(3)]

    # G accumulator: [96, 2, 96] f32, head-pairs packed, lives all of phase A
    G = pG.tile([96, 2, HEADS // 2 * HC], F32)

    last_pe = last_act = last_dve = last_pool = None

    # per-strip gram work emitted one strip late for pipelining
    pend_gram = None

    # gram computed TRANSPOSED (G'[d, c] = sum_n k[d] q[c]) so phase B needs
    # only one transpose per head: scale by rk on partitions, transpose,
    # then scale by rq*temp during the psum evac.
    def emit_gram(qkT, s):
        nonlocal last_pe
        for r in range(H_STRIP):
            first = (s == 0 and r == 0)
            last = (s == NSTRIP - 1 and r == H_STRIP - 1)
            for hp in range(2):
                last_pe = nc.tensor.matmul(
                    G[:, hp, :], qkT[:, r, C + hp * 96:C + (hp + 1) * 96],
                    qkT[:, r, hp * 96:(hp + 1) * 96],
                    start=first, stop=last, skip_group_check=True)

    # ---------------- phase A: strips ----------------
    for s in range(NSTRIP):
        y0 = s * H_STRIP
        ytop = max(y0 - 1, 0)
        ybot = min(y0 + H_STRIP + 1, HW)  # exclusive
        # halo reuse: strips s>0 copy rows y0-1,y0 from the previous strip's
        # slab bottom; only rows y0+1..ybot-1 are computed fresh.
        if s == 0:
            cbase_x = 0                   # first computed x column
            wbase = 2 + 128               # slab write base
        else:
            cbase_x = (y0 + 1) * 128
            wbase = 2 + 2 * 128
        cols = ybot * 128 - cbase_x       # computed cols this strip

        prev_slabs = None if s == 0 else (slab8, slabv3, slabv4)
        slab8 = [slab_pool.tile([128, PAD], F8, name=f"slab8_{i}", tag=f"slab8_{i}")
                 for i in range(3)]
        slabv3 = slab_pool.tile([128, PAD], BF16, name="slabv3", tag="slabv3")
        slabv4 = slab_pool.tile([64, PAD], BF16, name="slabv4", tag="slabv4")
        if s < 2:
            for t_ in slab8 + [slabv3, slabv4]:
                nc.vector.memset(t_[:, 0:2], 0.0)
                nc.vector.memset(t_[:, PAD - 2:PAD], 0.0)
        if s == 0:
            for t_ in slab8 + [slabv3, slabv4]:
                nc.vector.memset(t_[:, 2:130], 0.0)  # top halo
        else:
            po8, pov3, pov4 = prev_slabs
            for dst, srcp in zip(slab8 + [slabv3, slabv4],
                                 po8 + [pov3, pov4]):
                last_pool = nc.gpsimd.tensor_copy(
                    out=dst[:, 2:258], in_=srcp[:, 2 + 16 * 128:2 + 18 * 128])

        x8_t = xpool.tile([96, 2, (H_STRIP + 1) * 128], F8, tag="x8")
        xa_t = xpool.tile([128, (H_STRIP + 1) * 128], BF16, tag="xa")
        xb_t = xpool.tile([64, (H_STRIP + 1) * 128], BF16, tag="xb")
        nc.sync.dma_start(out=x8_t[:, :, :cols], in_=E["x8"][:].rearrange(
            "p (t n) -> p t n", t=2)[:, :, cbase_x:ybot * 128])
        nc.sync.dma_start(out=xa_t[:, :cols], in_=E["xa"][:, cbase_x:ybot * 128])
        nc.sync.dma_start(out=xb_t[:, :cols], in_=E["xb"][:, cbase_x:ybot * 128])

        if s == NSTRIP - 1:
            for t_ in slab8 + [slabv3, slabv4]:
                nc.vector.memset(t_[:, 2 + (H_STRIP + 1) * 128:PAD - 2], 0.0)

        qk_sb = [qk_pool.tile([128, OUTC], BF16, name=f"qk{i}", tag=f"qk{i}")
                 for i in range(3)]

        # ---- tile emitters: A-tiles drain on Act (1024-wide), B on DVE ----
        ATW = 1024  # A-tile width: 2 psum banks, one wide evac
        ntiles = (cols + ATW - 1) // ATW

        def emit_a(ci, t):
            nonlocal last_act
            base = t * ATW
            w = min(ATW, cols - base)
            if w <= 0:
                return
            if ci < 3:
                ps = pQkv.tile([128, ATW], F32, tag="pqkv", name="psa")
                for j in range(0, w, 512):
                    wj = min(512, w - j)
                    nc.tensor.matmul(
                        ps[:, j:j + wj], w1qk8[:, :, ci * 128:(ci + 1) * 128],
                        x8_t[:, :, base + j:base + j + wj],
                        start=True, stop=True, perf_mode=DR)
                last_act = evac_act(
                    slab8[ci][:, wbase + base: wbase + base + w], ps[:, :w])
            else:
                mb_, msz, slab = [(0, 128, slabv3), (128, 64, slabv4)][ci - 3]
                ps = pQkv.tile([msz, ATW], F32, tag="pqkv", name="psv")
                for j in range(0, w, 512):
                    wj = min(512, w - j)
                    nc.tensor.matmul(ps[:, j:j + wj], w1va[:, mb_:mb_ + msz],
                                     xa_t[:, base + j:base + j + wj],
                                     start=True, stop=False)
                    nc.tensor.matmul(ps[:, j:j + wj], w1vb[:, mb_:mb_ + msz],
                                     xb_t[:, base + j:base + j + wj],
                                     start=False, stop=True)
                last_act = evac_act(slab[:, wbase + base: wbase + base + w],
                                    ps[:, :w])

        def emit_b(bi, nt):
            nonlocal last_act, last_dve, last_pe
            if bi < 3:  # qk dw chunk bi
                i = bi
                ps = pDw.tile([128, 512], F32, tag="pdw", name="psd")
                for p, (dA, dB) in enumerate(QK_PAIRS):
                    rhs = _shift_ap(slab8[i], OBASE + nt * 512 + dA, dB - dA, 512)
                    nc.tensor.matmul(ps[:], dwqk8[:, i, p, :, :], rhs,
                                     start=(p == 0), stop=(p == 3),
                                     perf_mode=DR)
                dst = qk_sb[i][:, nt * 512:(nt + 1) * 512]
                # evac + center tap: dst = w_c * slab8_center + psum (DVE stt)
                o = OBASE + nt * 512
                last_dve = nc.vector.scalar_tensor_tensor(
                    out=dst, in0=slab8[i][:, o:o + 512],
                    scalar=wc_qk[:, i:i + 1], in1=ps[:],
                    op0=ALU.mult, op1=ALU.add)
            else:  # v dw odd taps: bi==3 -> chunk3, bi==4 -> chunk4
                vb = vbar_a if bi == 3 else vbar_b
                rd = rdiag3 if bi == 3 else rdiag4
                csz = 128 if bi == 3 else 64
                slab = slabv3 if bi == 3 else slabv4
                ps = pDw.tile([csz, 512], F32, tag="pdw", name="psd")
                for t, (dy, dx) in enumerate(V3_PE_TAPS):
                    d = dy * 128 + dx
                    o = OBASE + nt * 512 + d
                    last_pe = nc.tensor.matmul(
                        ps[:], rd[:, t * csz:(t + 1) * csz],
                        slab[:, o:o + 512],
                        start=(t == 0), stop=(t == 5))
                dst = vb[:, y0 * 128 + nt * 512: y0 * 128 + (nt + 1) * 512]
                last_act = evac_act(dst, ps[:])

        # ---- zipper: interleave A (1x1) and B (dw) tile emission ----
        A = [(ci, t) for ci in range(5) for t in range(ntiles)]
        B = [(bi, nt) for bi in range(5) for nt in range(OUTC // 512)]
        DELAY = 3
        ai = bi_ = 0
        emitted_gram = False
        while ai < len(A) or bi_ < len(B):
            if ai < len(A):
                emit_a(*A[ai]); ai += 1
            if ai >= len(A) and not emitted_gram and pend_gram is not None:
                emit_gram(*pend_gram)
                pend_gram = None
                emitted_gram = True
            if ai >= DELAY or ai >= len(A):
                if bi_ < len(B) and (bi_ < (ai - DELAY) + 1 or ai >= len(A)):
                    emit_b(*B[bi_]); bi_ += 1
        if pend_gram is not None:
            emit_gram(*pend_gram)
            pend_gram = None

        # --- qk x-edge fixups (6 odd taps wrap at x=0/127) on DVE ---
        for i in range(3):
            d3 = qk_sb[i].rearrange("p (r x) -> p r x", x=128)
            dst_c0 = d3[:, :, 0:1]
            dst_c127 = d3[:, :, 127:128]
            for dy in (-1, 0, 1):
                t_m1 = V3_PE_TAPS.index((dy, -1))
                t_p1 = V3_PE_TAPS.index((dy, 1))
                a0 = (1 + dy) * 128 + 1
                src0 = slab8[i][:, a0:a0 + OUTC].rearrange(
                    "p (r x) -> p r x", x=128)[:, :, 0:1]
                last_dve = nc.vector.scalar_tensor_tensor(
                    out=dst_c0, in0=src0,
                    scalar=wneg_qk[:, i * 6 + t_m1: i * 6 + t_m1 + 1],
                    in1=dst_c0, op0=ALU.mult, op1=ALU.add)
                a = (dy + 1) * 128 + 4
                src1 = slab8[i][:, a:a + OUTC].rearrange(
                    "p (r x) -> p r x", x=128)[:, :, 126:127]
                last_dve = nc.vector.scalar_tensor_tensor(
                    out=dst_c127, in0=src1,
                    scalar=wneg_qk[:, i * 6 + t_p1: i * 6 + t_p1 + 1],
                    in1=dst_c127, op0=ALU.mult, op1=ALU.add)

        # --- norms: sum of squares per channel on Act ---
        sq_scr = vmul_pool.tile([128, OUTC], BF16, tag="vp0", name="sq_scr")
        for i in range(3):
            last_act = nc.scalar.activation(
                out=sq_scr[:], in_=qk_sb[i][:], func=AF.Square,
                accum_out=nsq[i][:, s:s + 1])

        # --- qk transpose via DMA XBAR into [x, r, ch] ---
        qkT = qkT_pool.tile([128, H_STRIP, 2 * C], BF16, tag="qkT")
        for i in range(3):
            nc.sync.dma_start_transpose(
                out=qkT[:, :, i * 128:(i + 1) * 128], in_=qk_sb[i][:])
        pend_gram = (qkT, s)

        # --- v even taps: mul (4x) into scratch, add (2x) in-place to vbar ---
        va_sl = vbar_a[:, y0 * 128: y0 * 128 + OUTC]
        vb_sl = vbar_b[:, y0 * 128: y0 * 128 + OUTC]
        for ci, (slab, wt, sl, csz) in enumerate(
                [(slabv3, wtaps3, va_sl, 128), (slabv4, wtaps4, vb_sl, 64)]):
            for j, dy in enumerate((-1, 0, 1)):
                o = OBASE + dy * 128
                pr = vmul_pool.tile([csz, OUTC], BF16, tag=f"vp{j % 2}",
                                    name="pr")
                last_dve = nc.vector.tensor_scalar_mul(
                    pr[:], slab[:, o:o + OUTC], wt[:, j:j + 1])
                last_dve = nc.vector.tensor_add(sl, sl, pr[:])

        # --- v fixups (PE odd taps wrap) on DVE ---
        for slab, wneg, dst, csz in [(slabv3, wneg3, va_sl, 128),
                                     (slabv4, wneg4, vb_sl, 64)]:
            d3 = dst.rearrange("p (r x) -> p r x", x=128)
            for dy in (-1, 0, 1):
                t_m1 = V3_PE_TAPS.index((dy, -1))
                t_p1 = V3_PE_TAPS.index((dy, 1))
                a0 = (1 + dy) * 128 + 1
                src0 = slab[:, a0:a0 + OUTC].rearrange(
                    "p (r x) -> p r x", x=128)[:, :, 0:1]
                last_dve = nc.vector.scalar_tensor_tensor(
                    out=d3[:, :, 0:1], in0=src0,
                    scalar=wneg[:, t_m1:t_m1 + 1],
                    in1=d3[:, :, 0:1], op0=ALU.mult, op1=ALU.add)
                a = (dy + 1) * 128 + 4
                src1 = slab[:, a:a + OUTC].rearrange(
                    "p (r x) -> p r x", x=128)[:, :, 126:127]
                last_dve = nc.vector.scalar_tensor_tensor(
                    out=d3[:, :, 127:128], in0=src1,
                    scalar=wneg[:, t_p1:t_p1 + 1],
                    in1=d3[:, :, 127:128], op0=ALU.mult, op1=ALU.add)

    # tail gram
    if pend_gram is not None:
        emit_gram(*pend_gram)
        pend_gram = None

    # ---------------- phase B ----------------
    Gsb = small.tile([96, 2, 96], F32, tag="gsb")
    last_act = nc.scalar.copy(out=Gsb[:], in_=G[:])

    rn = []
    for i in range(3):
        tot = small.tile([128, 1], F32, tag=f"tot{i}")
        nc.vector.tensor_reduce(out=tot[:], in_=nsq[i][:], axis=AX.X, op=ALU.add)
        rt = small.tile([128, 1], F32, tag=f"rt{i}")
        nc.scalar.sqrt(out=rt[:], in_=tot[:])
        rr = small.tile([128, 1], F32, tag=f"rr{i}")
        nc.vector.reciprocal(out=rr[:], in_=rt[:])
        rn.append(rr)

    def gather_head(dst, global_base):
        done = 0
        g = global_base
        while done < HC:
            oc, off = g // 128, g % 128
            take = min(HC - done, 128 - off)
            nc.sync.dma_start(out=dst[done:done + take, :],
                              in_=rn[oc][off:off + take, :])
            done += take
            g += take

    mh_sb = []
    for h in range(HEADS):
        hp, off = h // 2, (h % 2) * HC
        rq = small.tile([HC, 1], F32, tag="rq")
        gather_head(rq, h * HC)
        rk = small.tile([HC, 1], F32, tag="rk")
        gather_head(rk, C + h * HC)
        rqt = small.tile([HC, 1], F32, tag="rqt")
        nc.vector.tensor_mul(rqt[:], rq[:], temps[:, h:h + 1])
        g_h = small.tile([HC, HC], F32, tag="gh")
        nc.sync.dma_start(out=g_h[:], in_=Gsb[off:off + HC, hp, off:off + HC])
        # g_h is G'[d, c]; scale rows by rk, transpose, scale by rq*temp.
        z1 = small.tile([HC, HC], F32, tag="z1")
        nc.vector.tensor_scalar_mul(z1[:], g_h[:], rk[:])
        z1T_ps = pT.tile([HC, HC], F32, tag="ptz")
        nc.tensor.transpose(z1T_ps[:], z1[:], ident48[:])
        z = small.tile([HC, HC], F32, tag="z")
        nc.scalar.activation(out=z[:], in_=z1T_ps[:], func=AF.Copy, scale=rqt[:])
        # |z| <= temperature (cosine-sim gram), so exp needs no max-shift
        e = small.tile([HC, HC], F32, tag="e")
        nc.scalar.activation(out=e[:], in_=z[:], func=AF.Exp, scale=1.0)
        ssum = small.tile([HC, 1], F32, tag="ssum")
        nc.vector.reduce_sum(ssum[:], e[:], AX.X)
        rs = small.tile([HC, 1], F32, tag="rs")
        nc.vector.reciprocal(rs[:], ssum[:])
        attn = small.tile([HC, HC], BF16, tag="attn")
        last_dve = nc.vector.tensor_scalar_mul(attn[:], e[:], rs[:])
        mh = pDw.tile([HC, C], F32, tag="pdw")
        nc.tensor.matmul(mh[:], attn[:], wpTh[:, h * C:(h + 1) * C],
                         start=True, stop=True)
        msb = small.tile([HC, C], BF16, tag=f"msb{h}")
        nc.scalar.copy(out=msb[:], in_=mh[:])
        mh_sb.append(msb)

    MTa = singles.tile([128, C], BF16)
    MTb = singles.tile([64, C], BF16)
    nc.sync.dma_start(out=MTa[0:48, :], in_=mh_sb[0][:])
    nc.sync.dma_start(out=MTa[48:96, :], in_=mh_sb[1][:])
    nc.sync.dma_start(out=MTa[96:128, :], in_=mh_sb[2][0:32, :])
    nc.sync.dma_start(out=MTb[0:16, :], in_=mh_sb[2][32:48, :])
    i_m = nc.sync.dma_start(out=MTb[16:64, :], in_=mh_sb[3][:])
    terminals.append(i_m)

    # ---------------- phase C: out = blockdiag-attn-proj @ vbar ----------------
    actx.close()  # free phase-A SBUF for wide output staging
    outp = ctx.enter_context(tc.tile_pool(name="outp", bufs=3))
    pC0 = ctx.enter_context(tc.tile_pool(name="pC0", bufs=2, space="PSUM"))
    pC1 = ctx.enter_context(tc.tile_pool(name="pC1", bufs=2, space="PSUM"))
    BLK = 1024
    for blk in range(N // BLK):
        ps0 = pC0.tile([128, BLK], F32, tag="pc0")
        ps1 = pC1.tile([64, BLK], F32, tag="pc1")
        for j in range(BLK // 512):
            sl = slice(blk * BLK + j * 512, blk * BLK + (j + 1) * 512)
            jsl = slice(j * 512, (j + 1) * 512)
            nc.tensor.matmul(ps0[:, jsl], MTa[:, 0:128], vbar_a[:, sl], start=True, stop=False)
            nc.tensor.matmul(ps0[:, jsl], MTb[:, 0:128], vbar_b[:, sl], start=False, stop=True)
            nc.tensor.matmul(ps1[:, jsl], MTa[:, 128:192], vbar_a[:, sl], start=True, stop=False)
            last_pe = nc.tensor.matmul(ps1[:, jsl], MTb[:, 128:192], vbar_b[:, sl],
                                       start=False, stop=True)
        o0 = outp.tile([128, BLK], BF16, tag="o0")
        o1 = outp.tile([64, BLK], BF16, tag="o1")
        if blk % 2 == 0:
            last_act = nc.scalar.copy(out=o0[:], in_=ps0[:])
            last_dve = nc.vector.tensor_copy(out=o1[:], in_=ps1[:])
        else:
            last_dve = nc.vector.tensor_copy(out=o0[:], in_=ps0[:])
            last_act = nc.scalar.copy(out=o1[:], in_=ps1[:])
        i0 = nc.sync.dma_start(out=E["out"][0:128, blk * BLK:(blk + 1) * BLK], in_=o0[:])
        i1 = nc.gpsimd.dma_start(out=E["out"][128:192, blk * BLK:(blk + 1) * BLK], in_=o1[:])
        terminals.append(i0)
        terminals.append(i1)

    terminals.append(last_pe)
    terminals.append(last_act)
    terminals.append(last_dve)
    terminals.append(last_pool)


# ----------------------------------------------------------------------------
# Public entry point: full inputs -> full output, 8-way data-parallel over
# batch across NeuronCores 0-7.
# ----------------------------------------------------------------------------

def kernel(x, w_qkv, w_dw, temperature, w_proj):
    from concourse.bass_utils import run_bass_kernel_spmd

    x = np.asarray(x, np.float32)
    B = x.shape[0]
    assert x.shape == (8, C, HW, HW), x.shape

    nc = bass.Bass()
    build(nc)

    wmaps = prep_weights(w_qkv, w_dw, temperature, w_proj)
    xmaps = prep_x(x)
    in_maps = [{**wmaps, **xm} for xm in xmaps]

    res = run_bass_kernel_spmd(nc, in_maps, core_ids=list(range(8)))
    out = np.stack([np.asarray(res.results[b]["out"]).astype(np.float32)
                    .reshape(C, HW, HW) for b in range(B)])
    return out

